# revision 1
# baseline (speedup 1.0000x reference)
"""Trainium2 Bass kernel for nn_DensityRatioEstimator (InfoNCE-style Cauchy-kernel loss).

Math: logits[i,j] = -log(1 + ||z_y_i - z_x_j||^2). All six outputs are scalar
reductions of the 8192x8192 logit matrix. Key identities used on device:
    exp(logit)     = 1/(1+d2)  = r      (logsumexp needs no max-subtraction: r <= 1)
    sigmoid(logit) = 1/(2+d2)  = r/(1+r) ~= r - r^2 + r^3 ...
so the slab work per core is: one K=128 fp32 matmul producing v = x2_j - 2*y_i.x_j
(PSUM), ACT pass Ln(v + (1+y2_i)) with fused row-accumulate, ACT pass Exp(-L)=r
with fused row-accumulate, and one DVE scalar_tensor_tensor (r-1)*r with fused
row-accumulate. Diagonal terms are recomputed exactly from row-major shards and
subtracted on the host, which also applies a per-row moment estimate for the
dropped r^3 term and combines all per-core partials in float64.

Sharding: rows of z_y across 8 cores (1024 rows each), z_x replicated.
"""

import numpy as np

N, D = 8192, 64
NCORES = 8
ROWS = N // NCORES          # 1024 z_y rows per core
RB = ROWS // 128            # 8 row-blocks of 128 rows
CHUNK = 2048                # columns per PSUM tile (4 banks)
CK = N // CHUNK             # 4 column chunks
NCOLS = RB * CK             # 32 accumulator columns per core

_PROGRAM = None


def _build_program():
    import concourse.bacc as bacc
    import concourse.mybir as mybir
    import concourse.tile as tile

    f32 = mybir.dt.float32
    AF = mybir.ActivationFunctionType
    OP = mybir.AluOpType

    # Bacc (not plain Bass): its compile() pass pipeline splits multi-sem waits
    # (generate_event_semaphores) — required for fp32 self-loading matmuls whose
    # S3_LW struct takes a single wait — and inserts ACT table loads.
    nc = bacc.Bacc("TRN2", target_bir_lowering=False, debug=False)

    xT = nc.dram_tensor("xT", [D, N], f32, kind="ExternalInput")
    yT = nc.dram_tensor("yT", [D, ROWS], f32, kind="ExternalInput")
    yrows = nc.dram_tensor("yrows", [128, RB * D], f32, kind="ExternalInput")
    xrows = nc.dram_tensor("xrows", [128, RB * D], f32, kind="ExternalInput")
    o_accL = nc.dram_tensor("o_accL", [128, NCOLS], f32, kind="ExternalOutput")
    o_accR = nc.dram_tensor("o_accR", [128, NCOLS], f32, kind="ExternalOutput")
    o_accC = nc.dram_tensor("o_accC", [128, NCOLS], f32, kind="ExternalOutput")
    o_small = nc.dram_tensor("o_small", [128, 3], f32, kind="ExternalOutput")

    with tile.TileContext(nc) as tc:
        with (
            tc.tile_pool(name="const", bufs=1) as const,
            tc.tile_pool(name="work", bufs=3) as work,
            tc.tile_pool(name="psum", bufs=2, space="PSUM") as psum,
        ):
            # Moving operand, one tile per column chunk so each matmul waits on
            # few producers: rows 0-63 = xT, rows 64-127 = xT^2 (squared in place).
            rp_cks = []
            for ck in range(CK):
                rp = const.tile([128, CHUNK], f32, tag=f"rp{ck}")
                cs = slice(ck * CHUNK, (ck + 1) * CHUNK)
                nc.sync.dma_start(out=rp[0:64, :], in_=xT[:, cs])
                nc.sync.dma_start(out=rp[64:128, :], in_=xT[:, cs])
                nc.vector.tensor_mul(rp[64:128, :], rp[64:128, :], rp[64:128, :])
                rp_cks.append(rp)

            # Stationary operand per row-block: rows 0-63 = -2*yT_rb, rows 64-127 = 1.
            wsb = const.tile([128, ROWS], f32)
            ytmp = const.tile([64, ROWS], f32)
            nc.sync.dma_start(out=ytmp[:, :], in_=yT[:, :])
            nc.vector.tensor_scalar_mul(wsb[0:64, :], ytmp[:, :], -2.0)
            nc.vector.memset(wsb[64:128, :], 1.0)

            # Row-major shards for y2 bias + exact diagonal terms.
            yr = const.tile([128, RB, D], f32)
            xr = const.tile([128, RB, D], f32)
            nc.sync.dma_start(out=yr[:, :, :], in_=yrows[:, :].rearrange("p (rb d) -> p rb d", d=D))
            nc.sync.dma_start(out=xr[:, :, :], in_=xrows[:, :].rearrange("p (rb d) -> p rb d", d=D))

            # bias[:, rb] = 1 + sum_d y^2
            bias = const.tile([128, RB], f32)
            sq_scr = const.tile([128, RB, D], f32)
            y2t = const.tile([128, RB], f32)
            nc.vector.tensor_mul(sq_scr[:, :, :], yr[:, :, :], yr[:, :, :])
            nc.vector.tensor_reduce(
                out=y2t[:, :], in_=sq_scr[:, :, :], axis=mybir.AxisListType.X, op=OP.add
            )
            nc.vector.tensor_scalar_add(bias[:, :], y2t[:, :], 1.0)

            # Exact diagonal: d2ii = sum_d (y-x)^2 per row.
            diff = const.tile([128, RB, D], f32)
            nc.vector.tensor_sub(diff[:, :, :], yr[:, :, :], xr[:, :, :])
            sqd = const.tile([128, RB, D], f32)
            nc.vector.tensor_mul(sqd[:, :, :], diff[:, :, :], diff[:, :, :])
            d2ii = const.tile([128, RB], f32)
            nc.vector.tensor_reduce(out=d2ii[:, :], in_=sqd[:, :, :], axis=mybir.AxisListType.X, op=OP.add)

            # Diagonal terms via ACT only (reciprocal/ttr are not supported by
            # this runtime): ln(1+d2), r_ii = exp(-ln(1+d2)), s_ii = exp(-ln(2+d2)).
            small = const.tile([128, 3], f32)
            lnpos = const.tile([128, RB], f32)
            nc.scalar.activation(
                lnpos[:, :], d2ii[:, :], AF.Ln, bias=1.0, scale=1.0, accum_out=small[:, 0:1]
            )
            rhat = const.tile([128, RB], f32)
            nc.scalar.activation(rhat[:, :], lnpos[:, :], AF.Exp, scale=-1.0)
            d2p2 = const.tile([128, RB], f32)
            nc.vector.tensor_scalar_add(d2p2[:, :], d2ii[:, :], 2.0)
            ln2t = const.tile([128, RB], f32)
            nc.scalar.activation(ln2t[:, :], d2p2[:, :], AF.Ln)
            shat = const.tile([128, RB], f32)
            nc.scalar.activation(shat[:, :], ln2t[:, :], AF.Exp, scale=-1.0, accum_out=small[:, 1:2])

            # Main slab: 8 row-blocks x 4 column chunks of [128, 2048].
            accL = const.tile([128, NCOLS], f32)
            accR = const.tile([128, NCOLS], f32)
            accC = const.tile([128, NCOLS], f32)
            for rb in range(RB):
                w_ap = wsb[:, rb * 128 : (rb + 1) * 128]
                for ck in range(CK):
                    col = rb * CK + ck
                    v = psum.tile([128, CHUNK], f32, tag="v")
                    for j in range(4):
                        nc.tensor.matmul(
                            out=v[:, j * 512 : (j + 1) * 512],
                            lhsT=w_ap,
                            rhs=rp_cks[ck][:, j * 512 : (j + 1) * 512],
                            start=True,
                            stop=True,
                        )
                    L = work.tile([128, CHUNK], f32, tag="L")
                    nc.scalar.activation(
                        L[:, :], v[:, :], AF.Ln,
                        bias=bias[:, rb : rb + 1], scale=1.0,
                        accum_out=accL[:, col : col + 1],
                    )
                    r = work.tile([128, CHUNK], f32, tag="r")
                    nc.scalar.activation(
                        r[:, :], L[:, :], AF.Exp, scale=-1.0,
                        accum_out=accR[:, col : col + 1],
                    )
                    scr = work.tile([128, CHUNK], f32, tag="scr")
                    nc.vector.scalar_tensor_tensor(
                        out=scr[:, :], in0=r[:, :], scalar=1.0, in1=r[:, :],
                        op0=OP.subtract, op1=OP.mult,
                        accum_out=accC[:, col : col + 1],
                    )

            # Per-row logsumexp term: ln(sum_j r - r_ii).
            Rall = const.tile([128, RB], f32)
            nc.vector.tensor_reduce(
                out=Rall[:, :],
                in_=accR[:, :].rearrange("p (rb ck) -> p rb ck", ck=CK),
                axis=mybir.AxisListType.X,
                op=OP.add,
            )
            Roff = const.tile([128, RB], f32)
            nc.vector.tensor_sub(Roff[:, :], Rall[:, :], rhat[:, :])
            lnr_t = const.tile([128, RB], f32)
            nc.scalar.activation(lnr_t[:, :], Roff[:, :], AF.Ln, accum_out=small[:, 2:3])

            nc.sync.dma_start(out=o_accL[:, :], in_=accL[:, :])
            nc.sync.dma_start(out=o_accR[:, :], in_=accR[:, :])
            nc.sync.dma_start(out=o_accC[:, :], in_=accC[:, :])
            nc.sync.dma_start(out=o_small[:, :], in_=small[:, :])

    nc.finalize()
    return nc


_RUNNER = None


def _make_runner():
    """Cached jitted shard_map runner over the 8 cores (the multi-core branch
    of bass2jax.run_bass_via_pjrt, kept so repeat calls don't re-jit)."""
    global _PROGRAM, _RUNNER
    if _RUNNER is not None:
        return _RUNNER
    import jax
    import numpy as _np
    from jax.sharding import Mesh, PartitionSpec
    from jax.experimental.shard_map import shard_map
    import concourse.mybir as mybir
    from concourse import bass2jax

    if _PROGRAM is None:
        _PROGRAM = _build_program()
    nc = _PROGRAM
    bass2jax.install_neuronx_cc_hook()

    partition_name = nc.partition_id_tensor.name if nc.partition_id_tensor else None
    in_names, out_names, out_avals, zero_shapes = [], [], [], []
    for alloc in nc.m.functions[0].allocations:
        if not isinstance(alloc, mybir.MemoryLocationSet):
            continue
        name = alloc.memorylocations[0].name
        if alloc.kind == "ExternalInput":
            if name != partition_name:
                in_names.append(name)
        elif alloc.kind == "ExternalOutput":
            out_names.append(name)
            shape = tuple(alloc.tensor_shape)
            dtype = mybir.dt.np(alloc.dtype)
            out_avals.append(jax.core.ShapedArray(shape, dtype))
            zero_shapes.append((shape, dtype))
    n_params = len(in_names)
    n_outs = len(out_avals)
    all_names = in_names + out_names
    if partition_name is not None:
        all_names = all_names + [partition_name]
    donate = tuple(range(n_params, n_params + n_outs))

    def _body(*args):
        operands = list(args)
        if partition_name is not None:
            operands.append(bass2jax.partition_id_tensor())
        outs = bass2jax._bass_exec_p.bind(
            *operands,
            out_avals=tuple(out_avals),
            in_names=tuple(all_names),
            out_names=tuple(out_names),
            lowering_input_output_aliases=(),
            sim_require_finite=True,
            sim_require_nnan=True,
            nc=nc,
        )
        return tuple(outs)

    devices = jax.devices()[:NCORES]
    mesh = Mesh(_np.asarray(devices), ("core",))
    in_specs = (PartitionSpec("core"),) * (n_params + n_outs)
    out_specs = (PartitionSpec("core"),) * n_outs
    sharded = jax.jit(
        shard_map(_body, mesh=mesh, in_specs=in_specs, out_specs=out_specs, check_rep=False),
        donate_argnums=donate,
        keep_unused=True,
    )
    _RUNNER = (sharded, in_names, out_names, out_avals, zero_shapes)
    return _RUNNER


def _prepare_concat_inputs(z_x, z_y):
    """Shard + lay out host inputs: concat of per-core input sets along axis 0."""
    xT = np.ascontiguousarray(z_x.T)
    per_core = []
    for c in range(NCORES):
        ys = z_y[c * ROWS : (c + 1) * ROWS]
        xs = z_x[c * ROWS : (c + 1) * ROWS]
        per_core.append(
            {
                "xT": xT,
                "yT": np.ascontiguousarray(ys.T),
                "yrows": np.ascontiguousarray(
                    ys.reshape(RB, 128, D).transpose(1, 0, 2).reshape(128, RB * D)
                ),
                "xrows": np.ascontiguousarray(
                    xs.reshape(RB, 128, D).transpose(1, 0, 2).reshape(128, RB * D)
                ),
            }
        )
    _, in_names, _, _, _ = _make_runner()
    return [
        np.concatenate([per_core[c][name] for c in range(NCORES)], axis=0)
        for name in in_names
    ]


def _execute(concat_in):
    """Run the cached executable; returns per-core results dicts."""
    sharded, in_names, out_names, out_avals, zero_shapes = _make_runner()
    zeros = [
        np.zeros((NCORES * s[0], *s[1:]), dt) for (s, dt) in zero_shapes
    ]
    out_arrs = sharded(*concat_in, *zeros)
    return [
        {
            name: np.asarray(out_arrs[i]).reshape(NCORES, *out_avals[i].shape)[c]
            for i, name in enumerate(out_names)
        }
        for c in range(NCORES)
    ]


def kernel(z_x, z_y):
    z_x = np.asarray(z_x, dtype=np.float32)
    z_y = np.asarray(z_y, dtype=np.float32)
    assert z_x.shape == (N, D) and z_y.shape == (N, D)

    results = _execute(_prepare_concat_inputs(z_x, z_y))

    # Host combine (float64): the unshard/all-reduce of per-core scalar partials.
    SL = SR = SC = P1 = P3 = P5 = 0.0
    corr3 = 0.0
    for c in range(NCORES):
        out = results[c]
        accL = out["o_accL"].astype(np.float64)
        accR = out["o_accR"].astype(np.float64)
        accC = out["o_accC"].astype(np.float64)
        small = out["o_small"].astype(np.float64)
        SL += accL.sum()
        SR += accR.sum()
        SC += accC.sum()
        P1 += small[:, 0].sum()
        P3 += small[:, 1].sum()
        P5 += small[:, 2].sum()
        # Per-row moment estimate of the dropped sum_j r^3 term:
        # R = sum r, Q = sum r^2 per row; sum r^3 ~= Q^2 / R.
        R_row = accR.reshape(128, RB, CK).sum(2)
        Q_row = R_row + accC.reshape(128, RB, CK).sum(2)
        corr3 += (Q_row * Q_row / R_row).sum()

    n = float(N)
    mean_pos = -P1 / n
    mean_neg = -(SL - P1) / (n * (n - 1))
    mean_sig_pos = P3 / n
    # sum sigmoid over full slab: sum r - sum r^2 + sum r^3(est); accC = sum(r^2 - r)
    S_sig_all = -SC + corr3
    mean_sig_neg = (S_sig_all - P3) / (n * (n - 1))
    log_baseline = 0.0
    loss = P1 / n + P5 / n - np.log(n - 1)

    return (
        np.float32(mean_pos),
        np.float32(mean_neg),
        np.float32(mean_sig_pos),
        np.float32(mean_sig_neg),
        np.float32(log_baseline),
        np.float32(loss),
    )



# revision 2
# speedup vs baseline: 2192.9425x; 2192.9425x over previous
"""Trainium2 Bass kernel for nn_DensityRatioEstimator (InfoNCE-style Cauchy-kernel loss).

Math: logits[i,j] = -log(1 + ||z_y_i - z_x_j||^2). All six outputs are scalar
reductions of the 8192x8192 logit matrix. Key identities used on device:
    exp(logit)     = 1/(1+d2)  = r      (logsumexp needs no max-subtraction: r <= 1)
    sigmoid(logit) = 1/(2+d2)  = r/(1+r) ~= r - r^2 + r^3 ...
Per core the slab work per [128, 4096] chunk is: one K=128 fp32 matmul producing
v = x2_j - 2*y_i.x_j (filling all 8 PSUM banks), ACT pass Ln(v + (1+y2_i)) with
fused row-accumulate, ACT pass Exp(-L)=r with fused row-accumulate, and one DVE
scalar_tensor_tensor (r-1)*r with fused row-accumulate. The kernel is ACT-bound:
2 transcendental passes over 8M elems/core at 1 elem/lane/cycle @ 1.2 GHz.
Diagonal terms are recomputed exactly from row-major shards; all per-core
reductions (including the r^3 moment correction sum exp(2*ln Q - ln R)) finish
on device so each core only ships a [128, 7] partial tile to the host, which
combines the 8 cores in float64.

Sharding: rows of z_y across 8 cores (1024 rows each), z_x replicated.

_build_program(reps=K) unrolls the whole body K times inside one NEFF so the
test harness can measure the marginal on-device execution time of one kernel
run, independent of the axon dispatch round-trip.
"""

import numpy as np

N, D = 8192, 64
NCORES = 8
ROWS = N // NCORES          # 1024 z_y rows per core
RB = ROWS // 128            # 8 row-blocks of 128 rows
CHUNK = 4096                # columns per PSUM tile (all 8 banks)
CK = N // CHUNK             # 2 column chunks
NCOLS = RB * CK             # 16 accumulator columns per core
OUTW = 7                    # SLr, SRr, SCr, P1, P3, P5, corr3

_PROGRAMS = {}
_RUNNERS = {}


def _build_program(reps=1):
    import concourse.bacc as bacc
    import concourse.mybir as mybir
    import concourse.tile as tile

    f32 = mybir.dt.float32
    AF = mybir.ActivationFunctionType
    OP = mybir.AluOpType

    # Bacc (not plain Bass): its compile() pass pipeline splits multi-sem waits
    # (generate_event_semaphores) — required for fp32 self-loading matmuls whose
    # S3_LW struct takes a single wait — and inserts ACT table loads.
    nc = bacc.Bacc("TRN2", target_bir_lowering=False, debug=False)

    xT = nc.dram_tensor("xT", [D, N], f32, kind="ExternalInput")
    yT = nc.dram_tensor("yT", [D, ROWS], f32, kind="ExternalInput")
    yrows = nc.dram_tensor("yrows", [128, RB * D], f32, kind="ExternalInput")
    xrows = nc.dram_tensor("xrows", [128, RB * D], f32, kind="ExternalInput")
    o_out = nc.dram_tensor("o_out", [128, OUTW], f32, kind="ExternalOutput")

    with tile.TileContext(nc) as tc:
        with (
            tc.tile_pool(name="io", bufs=1) as io,
            tc.tile_pool(name="setup", bufs=1) as setup,
            tc.tile_pool(name="work", bufs=2) as work,
            tc.tile_pool(name="psum", bufs=1, space="PSUM") as psum,
        ):
            for _rep in range(reps):
                # Moving operand, one tile per column chunk: rows 0-63 = xT,
                # rows 64-127 = xT^2 (squared in place).
                rp_cks = []
                for ck in range(CK):
                    rp = io.tile([128, CHUNK], f32, tag=f"rp{ck}")
                    cs = slice(ck * CHUNK, (ck + 1) * CHUNK)
                    nc.sync.dma_start(out=rp[0:64, :], in_=xT[:, cs])
                    nc.sync.dma_start(out=rp[64:128, :], in_=xT[:, cs])
                    nc.vector.tensor_mul(rp[64:128, :], rp[64:128, :], rp[64:128, :])
                    rp_cks.append(rp)

                # Stationary operand per row-block: rows 0-63 = -2*yT_rb, rows 64-127 = 1.
                wsb = io.tile([128, ROWS], f32, tag="wsb")
                ytmp = io.tile([64, ROWS], f32, tag="ytmp")
                nc.sync.dma_start(out=ytmp[:, :], in_=yT[:, :])
                nc.vector.tensor_scalar_mul(wsb[0:64, :], ytmp[:, :], -2.0)
                nc.vector.memset(wsb[64:128, :], 1.0)

                # Row-major shards for y2 bias + exact diagonal terms.
                yr = io.tile([128, RB, D], f32, tag="yr")
                xr = io.tile([128, RB, D], f32, tag="xr")
                nc.sync.dma_start(out=yr[:, :, :], in_=yrows[:, :].rearrange("p (rb d) -> p rb d", d=D))
                nc.sync.dma_start(out=xr[:, :, :], in_=xrows[:, :].rearrange("p (rb d) -> p rb d", d=D))

                # bias[:, rb] = 1 + sum_d y^2
                bias = setup.tile([128, RB], f32, tag="bias", bufs=1)
                sq_scr = setup.tile([128, RB, D], f32, tag="sq_scr", bufs=1)
                y2t = setup.tile([128, RB], f32, tag="y2t", bufs=1)
                nc.vector.tensor_mul(sq_scr[:, :, :], yr[:, :, :], yr[:, :, :])
                nc.vector.tensor_reduce(
                    out=y2t[:, :], in_=sq_scr[:, :, :], axis=mybir.AxisListType.X, op=OP.add
                )
                nc.vector.tensor_scalar_add(bias[:, :], y2t[:, :], 1.0)

                # Exact diagonal: d2ii = sum_d (y-x)^2 per row.
                diff = setup.tile([128, RB, D], f32, tag="diff", bufs=1)
                nc.vector.tensor_sub(diff[:, :, :], yr[:, :, :], xr[:, :, :])
                sqd = setup.tile([128, RB, D], f32, tag="sqd", bufs=1)
                nc.vector.tensor_mul(sqd[:, :, :], diff[:, :, :], diff[:, :, :])
                d2ii = setup.tile([128, RB], f32, tag="d2ii", bufs=1)
                nc.vector.tensor_reduce(out=d2ii[:, :], in_=sqd[:, :, :], axis=mybir.AxisListType.X, op=OP.add)

                # Per-core output partials: SLr, SRr, SCr, P1, P3, P5, corr3.
                osb = setup.tile([128, OUTW], f32, tag="osb", bufs=1)

                # Diagonal terms via ACT only (reciprocal/ttr are not supported
                # by this runtime): ln(1+d2), r_ii = exp(-ln(1+d2)),
                # s_ii = exp(-ln(2+d2)).
                lnpos = setup.tile([128, RB], f32, tag="lnpos", bufs=1)
                nc.scalar.activation(
                    lnpos[:, :], d2ii[:, :], AF.Ln, bias=1.0, scale=1.0, accum_out=osb[:, 3:4]
                )
                rhat = setup.tile([128, RB], f32, tag="rhat", bufs=1)
                nc.scalar.activation(rhat[:, :], lnpos[:, :], AF.Exp, scale=-1.0)
                d2p2 = setup.tile([128, RB], f32, tag="d2p2", bufs=1)
                nc.vector.tensor_scalar_add(d2p2[:, :], d2ii[:, :], 2.0)
                ln2t = setup.tile([128, RB], f32, tag="ln2t", bufs=1)
                nc.scalar.activation(ln2t[:, :], d2p2[:, :], AF.Ln)
                shat = setup.tile([128, RB], f32, tag="shat", bufs=1)
                nc.scalar.activation(shat[:, :], ln2t[:, :], AF.Exp, scale=-1.0, accum_out=osb[:, 4:5])

                # Main slab: 8 row-blocks x 2 column chunks of [128, 4096].
                accL = setup.tile([128, NCOLS], f32, tag="accL", bufs=1)
                accR = setup.tile([128, NCOLS], f32, tag="accR", bufs=1)
                accC = setup.tile([128, NCOLS], f32, tag="accC", bufs=1)
                for rb in range(RB):
                    w_ap = wsb[:, rb * 128 : (rb + 1) * 128]
                    for ck in range(CK):
                        col = rb * CK + ck
                        v = psum.tile([128, CHUNK], f32, tag="v")
                        for j in range(CHUNK // 512):
                            nc.tensor.matmul(
                                out=v[:, j * 512 : (j + 1) * 512],
                                lhsT=w_ap,
                                rhs=rp_cks[ck][:, j * 512 : (j + 1) * 512],
                                start=True,
                                stop=True,
                            )
                        L = work.tile([128, CHUNK], f32, tag="L")
                        nc.scalar.activation(
                            L[:, :], v[:, :], AF.Ln,
                            bias=bias[:, rb : rb + 1], scale=1.0,
                            accum_out=accL[:, col : col + 1],
                        )
                        r = work.tile([128, CHUNK], f32, tag="r")
                        nc.scalar.activation(
                            r[:, :], L[:, :], AF.Exp, scale=-1.0,
                            accum_out=accR[:, col : col + 1],
                        )
                        scr = work.tile([128, CHUNK], f32, tag="scr", bufs=1)
                        nc.vector.scalar_tensor_tensor(
                            out=scr[:, :], in0=r[:, :], scalar=1.0, in1=r[:, :],
                            op0=OP.subtract, op1=OP.mult,
                            accum_out=accC[:, col : col + 1],
                        )

                # Per-row sums over the ck chunks: R (sum r) and C (sum r^2 - r).
                Rall = setup.tile([128, RB], f32, tag="Rall", bufs=1)
                nc.vector.tensor_reduce(
                    out=Rall[:, :],
                    in_=accR[:, :].rearrange("p (rb ck) -> p rb ck", ck=CK),
                    axis=mybir.AxisListType.X,
                    op=OP.add,
                )
                Crow = setup.tile([128, RB], f32, tag="Crow", bufs=1)
                nc.vector.tensor_reduce(
                    out=Crow[:, :],
                    in_=accC[:, :].rearrange("p (rb ck) -> p rb ck", ck=CK),
                    axis=mybir.AxisListType.X,
                    op=OP.add,
                )

                # Per-row logsumexp term: ln(sum_j r - r_ii), accumulated to P5.
                Roff = setup.tile([128, RB], f32, tag="Roff", bufs=1)
                nc.vector.tensor_sub(Roff[:, :], Rall[:, :], rhat[:, :])
                lnr_t = setup.tile([128, RB], f32, tag="lnr_t", bufs=1)
                nc.scalar.activation(lnr_t[:, :], Roff[:, :], AF.Ln, accum_out=osb[:, 5:6])

                # Moment estimate of the dropped sum_j r^3 term, per row:
                # Q = sum r^2, R = sum r; sum r^3 ~= Q^2/R = exp(2 ln Q - ln R).
                Qrow = setup.tile([128, RB], f32, tag="Qrow", bufs=1)
                nc.vector.tensor_add(Qrow[:, :], Rall[:, :], Crow[:, :])
                lnQ = setup.tile([128, RB], f32, tag="lnQ", bufs=1)
                nc.scalar.activation(lnQ[:, :], Qrow[:, :], AF.Ln)
                lnRf = setup.tile([128, RB], f32, tag="lnRf", bufs=1)
                nc.scalar.activation(lnRf[:, :], Rall[:, :], AF.Ln)
                e2 = setup.tile([128, RB], f32, tag="e2", bufs=1)
                nc.vector.scalar_tensor_tensor(
                    out=e2[:, :], in0=lnQ[:, :], scalar=2.0, in1=lnRf[:, :],
                    op0=OP.mult, op1=OP.subtract,
                )
                c3t = setup.tile([128, RB], f32, tag="c3t", bufs=1)
                nc.scalar.activation(c3t[:, :], e2[:, :], AF.Exp, accum_out=osb[:, 6:7])

                # Column-reduce the big accumulators to one column each.
                nc.vector.tensor_reduce(
                    out=osb[:, 0:1],
                    in_=accL[:, :].rearrange("p (a b) -> p a b", a=1),
                    axis=mybir.AxisListType.X,
                    op=OP.add,
                )
                nc.vector.tensor_reduce(
                    out=osb[:, 1:2],
                    in_=accR[:, :].rearrange("p (a b) -> p a b", a=1),
                    axis=mybir.AxisListType.X,
                    op=OP.add,
                )
                nc.vector.tensor_reduce(
                    out=osb[:, 2:3],
                    in_=accC[:, :].rearrange("p (a b) -> p a b", a=1),
                    axis=mybir.AxisListType.X,
                    op=OP.add,
                )

                nc.sync.dma_start(out=o_out[:, :], in_=osb[:, :])

    nc.finalize()
    return nc


def _make_runner(reps=1):
    """Cached jitted shard_map runner over the 8 cores (the multi-core branch
    of bass2jax.run_bass_via_pjrt, kept so repeat calls don't re-jit)."""
    if reps in _RUNNERS:
        return _RUNNERS[reps]
    import jax
    import numpy as _np
    from jax.sharding import Mesh, PartitionSpec
    from jax.experimental.shard_map import shard_map
    import concourse.mybir as mybir
    from concourse import bass2jax

    if reps not in _PROGRAMS:
        _PROGRAMS[reps] = _build_program(reps)
    nc = _PROGRAMS[reps]
    bass2jax.install_neuronx_cc_hook()

    partition_name = nc.partition_id_tensor.name if nc.partition_id_tensor else None
    in_names, out_names, out_avals, zero_shapes = [], [], [], []
    for alloc in nc.m.functions[0].allocations:
        if not isinstance(alloc, mybir.MemoryLocationSet):
            continue
        name = alloc.memorylocations[0].name
        if alloc.kind == "ExternalInput":
            if name != partition_name:
                in_names.append(name)
        elif alloc.kind == "ExternalOutput":
            out_names.append(name)
            shape = tuple(alloc.tensor_shape)
            dtype = mybir.dt.np(alloc.dtype)
            out_avals.append(jax.core.ShapedArray(shape, dtype))
            zero_shapes.append((shape, dtype))
    n_params = len(in_names)
    n_outs = len(out_avals)
    all_names = in_names + out_names
    if partition_name is not None:
        all_names = all_names + [partition_name]
    donate = tuple(range(n_params, n_params + n_outs))

    def _body(*args):
        operands = list(args)
        if partition_name is not None:
            operands.append(bass2jax.partition_id_tensor())
        outs = bass2jax._bass_exec_p.bind(
            *operands,
            out_avals=tuple(out_avals),
            in_names=tuple(all_names),
            out_names=tuple(out_names),
            lowering_input_output_aliases=(),
            sim_require_finite=True,
            sim_require_nnan=True,
            nc=nc,
        )
        return tuple(outs)

    devices = jax.devices()[:NCORES]
    mesh = Mesh(_np.asarray(devices), ("core",))
    in_specs = (PartitionSpec("core"),) * (n_params + n_outs)
    out_specs = (PartitionSpec("core"),) * n_outs
    sharded = jax.jit(
        shard_map(_body, mesh=mesh, in_specs=in_specs, out_specs=out_specs, check_rep=False),
        donate_argnums=donate,
        keep_unused=True,
    )
    _RUNNERS[reps] = (sharded, in_names, out_names, out_avals, zero_shapes)
    return _RUNNERS[reps]


def _prepare_concat_inputs(z_x, z_y):
    """Shard + lay out host inputs (concat of per-core input sets along axis 0),
    then put them on device once with the core sharding so repeat executions
    don't re-pay the host->device transfer."""
    import jax
    import numpy as _np
    from jax.sharding import Mesh, PartitionSpec, NamedSharding

    xT = np.ascontiguousarray(z_x.T)
    per_core = []
    for c in range(NCORES):
        ys = z_y[c * ROWS : (c + 1) * ROWS]
        xs = z_x[c * ROWS : (c + 1) * ROWS]
        per_core.append(
            {
                "xT": xT,
                "yT": np.ascontiguousarray(ys.T),
                "yrows": np.ascontiguousarray(
                    ys.reshape(RB, 128, D).transpose(1, 0, 2).reshape(128, RB * D)
                ),
                "xrows": np.ascontiguousarray(
                    xs.reshape(RB, 128, D).transpose(1, 0, 2).reshape(128, RB * D)
                ),
            }
        )
    _, in_names, _, _, _ = _make_runner(1)
    concat = [
        np.concatenate([per_core[c][name] for c in range(NCORES)], axis=0)
        for name in in_names
    ]
    devices = jax.devices()[:NCORES]
    mesh = Mesh(_np.asarray(devices), ("core",))
    sh = NamedSharding(mesh, PartitionSpec("core"))
    dev = [jax.device_put(a, sh) for a in concat]
    for a in dev:
        a.block_until_ready()
    return dev


def _execute(concat_in, reps=1, fetch=True):
    """Run the cached executable; returns per-core results dicts (fetch=True)
    or the on-device output arrays (fetch=False, for timing)."""
    sharded, in_names, out_names, out_avals, zero_shapes = _make_runner(reps)
    zeros = [np.zeros((NCORES * s[0], *s[1:]), dt) for (s, dt) in zero_shapes]
    out_arrs = sharded(*concat_in, *zeros)
    if not fetch:
        return out_arrs
    return [
        {
            name: np.asarray(out_arrs[i]).reshape(NCORES, *out_avals[i].shape)[c]
            for i, name in enumerate(out_names)
        }
        for c in range(NCORES)
    ]


def kernel(z_x, z_y):
    z_x = np.asarray(z_x, dtype=np.float32)
    z_y = np.asarray(z_y, dtype=np.float32)
    assert z_x.shape == (N, D) and z_y.shape == (N, D)

    results = _execute(_prepare_concat_inputs(z_x, z_y))

    # Host combine (float64): the unshard/all-reduce of per-core scalar partials.
    SL = SC = P1 = P3 = P5 = corr3 = 0.0
    for c in range(NCORES):
        o = results[c]["o_out"].astype(np.float64)
        SL += o[:, 0].sum()
        SC += o[:, 2].sum()
        P1 += o[:, 3].sum()
        P3 += o[:, 4].sum()
        P5 += o[:, 5].sum()
        corr3 += o[:, 6].sum()

    n = float(N)
    mean_pos = -P1 / n
    mean_neg = -(SL - P1) / (n * (n - 1))
    mean_sig_pos = P3 / n
    # sum sigmoid over full slab: sum r - sum r^2 + sum r^3(est); SC = sum(r^2 - r)
    S_sig_all = -SC + corr3
    mean_sig_neg = (S_sig_all - P3) / (n * (n - 1))
    log_baseline = 0.0
    loss = P1 / n + P5 / n - np.log(n - 1)

    return (
        np.float32(mean_pos),
        np.float32(mean_neg),
        np.float32(mean_sig_pos),
        np.float32(mean_sig_neg),
        np.float32(log_baseline),
        np.float32(loss),
    )
